# revision 59
# baseline (speedup 1.0000x reference)
"""Causal multi-head attention block on 8 NeuronCores (Trainium2, Bass/Tile).

Reference computation (per batch b):
  Q = x @ W_Q + b_Q ; K = x @ W_K (b_K dropped: softmax shift-invariant)
  scores = Q K^T / sqrt(H); causal mask; probs = softmax(scores)
  out = (probs @ V) @ W_O + b_O   (b_V folded on host: probs rows sum to 1)

Sharding: core c -> batch c//2, head-group c%2 (6 of 12 heads). Host sums
the two head-group partials per batch and adds b_O + sum_nh b_V[n,h]*W_O.

Numerics: weights are host-folded x32 so fp8(e4m3) residual splits stay in
the normal range. All projections are fp8 DoubleRow matmuls (0.5 cy/row,
two 128-contractions per instr): Q/K = xh*Wh + (16 xls)*(Wh/16) (2-term,
residual error ~ W quant, damped ~0.43x by the small score scale);
V = 3-term (adds (xh/16)*(16 Wls)) since V error passes straight through.
Q is stored as an fp8 hi+lo pair (bias folded into hi via tensor_scalar_add,
lo via scalar_tensor_tensor); K as fp8 hi. Scores: one DoubleRow instr per
(head, ktile): slots (Kh, Qhi) + (Kh dup via 0-stride stationary, Qlo).
exp runs on Activation into bf16 left-packed per-head ets strips, merged
across ktiles sharing a PSUM tile; the last kt-group of each head PAIR
shares one [P, 2, *] exp call (5 exp instrs per head total). PV is flipped:
stationary = ets [k, q-block] (full 128x128 array), moving = V bf16
[k, 64] -> z [q, h] in one PSUM bank per q-tile, with per-head denominator
columns via a ones-vector moving operand. Normalize = one reciprocal + one
0-stride-broadcast multiply per q-tile; z^T via bf16 PE transposes
(identity is_transpose matmuls); out-proj bf16; bf16 output (host widens
to f32). Causal masking multiplies the 8 diagonal ets blocks by a host
upper-tri bf16 matrix on GPSIMD.

Scheduling: single warm PE stream ordered so the Activation engine (the
exp bottleneck, ~33us busy) starts by ~7us and never starves: pair-0
projections -> strip-0 scores interleaved with remaining projections ->
strip-1 scores -> per-q-tile PV + phase3 (normalize/transpose/out-proj)
pipelined, with strip-1 out-copies on Activation after its last exp.
DMA priority: pair-0 Q/K weights, x hi, x lo, QK pair-1/2 weights,
x half-1, V weights, x/16 (V-only input), W_O. 24 fp8 DoubleRow warm-up
matmuls carry the PE p-state ramp through the DMA prologue.

CoreSim exec time: 48327 ns (baseline fp32r kernel: 81439 ns).
Relative error vs f32 reference: 1.37e-2 (gate: 2e-2).
"""

import sys

sys.path.insert(0, "/opt/trn_rl_repo")

from contextlib import ExitStack

import numpy as np
import ml_dtypes

import concourse.bass as bass
import concourse.tile as tile
from concourse import bacc, mybir
from concourse.bass_utils import run_bass_kernel_spmd

B, S, D, N, H = 4, 1024, 768, 12, 64
NHC = 6            # heads per core
NPAIR = NHC // 2   # head pairs (2 heads -> 128 partitions)
HD = NHC * H       # 384 packed head dim per core
P = 128
NDT = D // P       # 6 d-tiles
NST = S // P       # 8 k-tiles
QB = 512           # strip width (q)
FW = 32.0          # host weight fold (power of 2)
F32 = mybir.dt.float32
F8 = mybir.dt.float8e4
BF = mybir.dt.bfloat16
DR = mybir.MatmulPerfMode.DoubleRow
EXP_SCALE = 1.0 / (np.sqrt(float(H)) * FW * FW)

QK_3T = False      # 3-term QK projections (False: 2-term, faster/less exact)

# wall column layout (fp8 weights, priority-ordered for DMA chunking)
def _wall_layout():
    cols = {}
    off = 0
    def put(name, w):
        nonlocal off
        cols[name] = (off, w)
        off += w
    # chunk 0: pair0 Q then K (two priority sub-chunks)
    for t in ("qh", "qhd") + (("qls",) if QK_3T else ()):
        put(t + "0", P)
    for t in ("kh", "khd") + (("kls",) if QK_3T else ()):
        put(t + "0", P)
    c0 = off
    # chunk 1: V (all pairs)
    for t in ("vh", "vhd", "vls"):
        put(t, HD)
    c1 = off
    # chunks 2, 3: pairs 1, 2
    for g in (1, 2):
        for t in ("qh", "kh", "qhd", "khd") + (("qls", "kls") if QK_3T else ()):
            put(t + str(g), P)
    return cols, (0, c0, c1, (c1 + off) // 2, off)

WALL_COLS, WALL_CHUNKS = _wall_layout()
WALL_W = WALL_CHUNKS[-1]

# ets strip segment offsets (left-packed live columns per ktile)
def _segs(strip):
    offs, off = [], 0
    nkt = 4 if strip == 0 else NST
    for kt in range(nkt):
        if strip == 0:
            live = QB - kt * P
        else:
            live = min(S - kt * P, QB)
        offs.append((off, live))
        off += live
    return offs, off

SEG0, LEN0 = _segs(0)
SEG1, LEN1 = _segs(1)
# exp merge groups: lists of (ktile list, psum cols used)
EXPG0 = [[0, 1], [2, 3]]
EXPG1 = [[0, 1], [2, 3], [4, 5], [6, 7]]

_CACHE = {}


def _build():
    nc = bacc.Bacc()
    wall_d = nc.declare_dram_parameter("wall", [D, WALL_W], F8, isOutput=False)
    xh_d = nc.declare_dram_parameter("xh", [D, S], F8, isOutput=False)
    xls_d = nc.declare_dram_parameter("xls", [D, S], F8, isOutput=False)
    xhd_d = nc.declare_dram_parameter("xhd", [D, S], F8, isOutput=False)
    wo_d = nc.declare_dram_parameter("wo", [HD, D], BF, isOutput=False)
    bq_d = nc.declare_dram_parameter("bq", [P, NPAIR], F32, isOutput=False)
    trid_d = nc.declare_dram_parameter("trid", [P, 2 * P], BF, isOutput=False)
    out_d = nc.declare_dram_parameter("out", [S, D], BF, isOutput=True)

    wall_r = wall_d[:].rearrange("(t p) c -> p t c", p=P)
    xh_r = xh_d[:].rearrange("(t p) s -> p t s", p=P)
    xls_r = xls_d[:].rearrange("(t p) s -> p t s", p=P)
    xhd_r = xhd_d[:].rearrange("(t p) s -> p t s", p=P)
    wo_r = wo_d[:].rearrange("(t p) d -> p t d", p=P)

    with tile.TileContext(nc) as tc, ExitStack() as ctx:
        consts = ctx.enter_context(tc.tile_pool(name="consts", bufs=1))
        persist = ctx.enter_context(tc.tile_pool(name="persist", bufs=1))
        znp = ctx.enter_context(tc.tile_pool(name="znp", bufs=2))
        zntp = ctx.enter_context(tc.tile_pool(name="zntp", bufs=2))
        outp = ctx.enter_context(tc.tile_pool(name="outp", bufs=2))
        smalls = ctx.enter_context(tc.tile_pool(name="smalls", bufs=2))

        wall = consts.tile([P, NDT, WALL_W], F8)
        xh = consts.tile([P, NDT, S], F8)
        xls = consts.tile([P, NDT, S], F8)
        xhd = consts.tile([P, NDT, S], F8, name="xhd")
        wo = consts.tile([P, NPAIR, D], BF)
        bq = consts.tile([P, NPAIR], F32)
        trid = consts.tile([P, 2 * P], BF)
        tri = trid[:, 0:P]
        ident = trid[:, P : 2 * P]

        def wslice(name, g, t0, nt, c0, cw):
            """stationary slice [P, nt, cw] of wall tensor `name` (pair g)."""
            base = WALL_COLS[name + str(g) if name[0] in "qk" else name][0]
            return wall[:, t0 : t0 + nt, base + c0 : base + c0 + cw]

        # ---- DMA priority order (all on SP/sync queue) ----
        def dma_wall_chunk(i):
            c0, c1 = WALL_CHUNKS[i], WALL_CHUNKS[i + 1]
            nc.sync.dma_start(out=wall[:, :, c0:c1], in_=wall_r[:, :, c0:c1])

        def dma_x_half(sb_t, dram_r, s2):
            nc.sync.dma_start(
                out=sb_t[:, :, s2 * QB : (s2 + 1) * QB],
                in_=dram_r[:, :, s2 * QB : (s2 + 1) * QB],
            )

        dma_wall_chunk(0)
        nc.sync.dma_start(out=bq, in_=bq_d[:])
        dma_x_half(xh, xh_r, 0)
        nc.sync.dma_start(out=trid, in_=trid_d[:])
        dma_x_half(xls, xls_r, 0)
        dma_wall_chunk(2)      # pair 1
        dma_wall_chunk(3)      # pair 2
        dma_x_half(xh, xh_r, 1)
        dma_x_half(xls, xls_r, 1)
        dma_wall_chunk(1)      # V weights
        dma_x_half(xhd, xhd_r, 0)
        nc.sync.dma_start(out=wo, in_=wo_r)
        dma_x_half(xhd, xhd_r, 1)

        # ---- persistent activations ----
        qT8 = persist.tile([P, NPAIR, 2, S], F8)   # dim2: hi/lo
        kT8 = persist.tile([P, NPAIR, S], F8)
        vA = persist.tile([P, NST, NHC, H], BF)
        ets0 = persist.tile([P, NHC, LEN0], BF)
        ets1 = persist.tile([P, NHC, LEN1], BF)
        ones = persist.tile([P, 1], BF)
        nc.gpsimd.memset(ones, 1.0)

        ps_sm = ctx.enter_context(tc.tile_pool(name="ps_sm", bufs=2, space="PSUM"))
        ps_z = ctx.enter_context(tc.tile_pool(name="ps_z", bufs=2, space="PSUM"))
        ps_m = ctx.enter_context(tc.tile_pool(name="ps_m", bufs=2, space="PSUM"))

        # PE warm-up: carries the p-state ramp during the DMA prologue.
        dums = consts.tile([P, 2, QB], F8)
        nc.gpsimd.memset(dums, 0.0)
        # preload the Exp activation table off the critical path
        scr = smalls.tile([P, 1], BF, name="scr")
        nc.scalar.activation(scr, ones, mybir.ActivationFunctionType.Exp)
        wps = ps_m.tile([P, QB], F32, name="warm", tag="m")
        for i in range(24):
            nc.tensor.matmul(
                wps, dums[:, :, 0:P], dums, start=(i == 0), stop=(i == 23),
                perf_mode=DR,
            )

        def proj_qk(g, s2):
            """Q and K projections for pair g, s-half s2 (fp8 DR)."""
            s0 = s2 * QB
            k_on_act = (g == 0 and s2 == 0)
            for wname, ap_hi_lo in (("q", True), ("k", False)):
                pp = ps_m.tile([P, QB], F32, tag="m")
                ktp = [(t, t + 2) for t in (0, 2, 4)]
                for t, _ in ktp:
                    nc.tensor.matmul(
                        pp, wslice(wname + "h", g, t, 2, 0, P),
                        xh[:, t : t + 2, s0 : s0 + QB],
                        start=(t == 0), stop=False, perf_mode=DR)
                for i, (t, _) in enumerate(ktp):
                    nc.tensor.matmul(
                        pp, wslice(wname + "hd", g, t, 2, 0, P),
                        xls[:, t : t + 2, s0 : s0 + QB],
                        start=False, stop=(not QK_3T and i == 2),
                        perf_mode=DR)
                if QK_3T:
                    for i, (t, _) in enumerate(ktp):
                        nc.tensor.matmul(
                            pp, wslice(wname + "ls", g, t, 2, 0, P),
                            xhd[:, t : t + 2, s0 : s0 + QB],
                            start=False, stop=(i == 2), perf_mode=DR)
                if ap_hi_lo:
                    nc.vector.tensor_scalar_add(
                        qT8[:, g, 0, s0 : s0 + QB], pp, bq[:, g : g + 1])
                    if not (g == 0 and s2 == 0):
                        nc.vector.scalar_tensor_tensor(
                            out=qT8[:, g, 1, s0 : s0 + QB], in0=pp,
                            scalar=bq[:, g : g + 1],
                            in1=qT8[:, g, 0, s0 : s0 + QB],
                            op0=mybir.AluOpType.add,
                            op1=mybir.AluOpType.subtract)
                elif k_on_act:
                    nc.scalar.copy(kT8[:, g, s0 : s0 + QB], pp)
                else:
                    nc.vector.tensor_copy(out=kT8[:, g, s0 : s0 + QB], in_=pp)

        def proj_v(st):
            """V projection for s-tile st -> vA bf16 (fp8 DR, 3-term)."""
            pp = ps_m.tile([P, HD], F32, tag="m")
            for t in (0, 2, 4):
                nc.tensor.matmul(
                    pp, xh[:, t : t + 2, st * P : (st + 1) * P],
                    wslice("vh", 0, t, 2, 0, HD),
                    start=(t == 0), stop=False, perf_mode=DR)
            for i, t in enumerate((0, 2, 4)):
                nc.tensor.matmul(
                    pp, xls[:, t : t + 2, st * P : (st + 1) * P],
                    wslice("vhd", 0, t, 2, 0, HD),
                    start=False, stop=False, perf_mode=DR)
            for i, t in enumerate((0, 2, 4)):
                nc.tensor.matmul(
                    pp, xhd[:, t : t + 2, st * P : (st + 1) * P],
                    wslice("vls", 0, t, 2, 0, HD),
                    start=False, stop=(i == 2), perf_mode=DR)
            nc.vector.tensor_copy(
                out=vA[:, st, :, :], in_=pp.rearrange("p (n h) -> p n h", n=NHC))

        def score_mm(sm_slice, h, kt, strip):
            """One score DR for (head, ktile) into the given psum slice."""
            g, e = divmod(h, 2)
            hp = e * H
            segs = SEG0 if strip == 0 else SEG1
            q0 = strip * QB
            live = segs[kt][1]
            o = QB - live
            kst = kT8[hp : hp + H, g, kt * P : (kt + 1) * P]
            if g == 0 and strip == 0:
                # pair0/strip0 runs hi-only (its Q-lo store is skipped to
                # shorten the prologue critical chain)
                nc.tensor.matmul(
                    sm_slice, kst,
                    qT8[hp : hp + H, g, 0, q0 + o : q0 + QB],
                    start=True, stop=True)
                return
            kst = kst.rearrange("p (o m) -> p o m", o=1).broadcast_to((H, 2, P))
            nc.tensor.matmul(
                sm_slice, kst,
                qT8[hp : hp + H, g, :, q0 + o : q0 + QB],
                start=True, stop=True, perf_mode=DR)

        def scores_head(h, strip):
            """Merged-group score DRs + exps for head h (all but last group)."""
            ets = ets0 if strip == 0 else ets1
            segs = SEG0 if strip == 0 else SEG1
            groups = (EXPG0 if strip == 0 else EXPG1)[:-1]
            for kts in groups:
                used = sum(segs[kt][1] for kt in kts)
                sm = ps_sm.tile([P, 2 * QB], F32, tag="sm")
                pcol = 0
                for kt in kts:
                    live = segs[kt][1]
                    score_mm(sm[:, pcol : pcol + live], h, kt, strip)
                    pcol += live
                seg0 = segs[kts[0]][0]
                nc.scalar.activation(
                    ets[:, h, seg0 : seg0 + used], sm[:, 0:used],
                    mybir.ActivationFunctionType.Exp, scale=EXP_SCALE)

        def scores_last_pair(g, strip):
            """Last kt-group of both heads of pair g: one shared exp."""
            ets = ets0 if strip == 0 else ets1
            segs = SEG0 if strip == 0 else SEG1
            kts = (EXPG0 if strip == 0 else EXPG1)[-1]
            used = sum(segs[kt][1] for kt in kts)
            sm = ps_sm.tile([P, 2, QB], F32, tag="sm")
            for i in range(2):
                pcol = 0
                for kt in kts:
                    live = segs[kt][1]
                    score_mm(sm[:, i, pcol : pcol + live], 2 * g + i, kt, strip)
                    pcol += live
            seg0 = segs[kts[0]][0]
            nc.scalar.activation(
                ets[:, 2 * g : 2 * g + 2, seg0 : seg0 + used], sm[:, :, 0:used],
                mybir.ActivationFunctionType.Exp, scale=EXP_SCALE)

        def masks_head(h, strip):
            ets = ets0 if strip == 0 else ets1
            segs = SEG0 if strip == 0 else SEG1
            dkts = range(4) if strip == 0 else range(4, 8)
            for kt in dkts:
                so = segs[kt][0]
                nc.gpsimd.tensor_mul(
                    ets[:, h, so : so + P], ets[:, h, so : so + P], tri)

        def pv_qtile(strip, j):
            """z psum for q-tile j of strip: all heads' PV + denominators."""
            ets = ets0 if strip == 0 else ets1
            segs = SEG0 if strip == 0 else SEG1
            q0 = strip * QB
            zp = ps_z.tile([P, 390], F32, tag="z")
            gq = q0 + j * P  # global q block start
            for h in range(NHC):
                kts = [kt for kt in range(len(segs))
                       if kt * P <= gq + P - 1 and True]
                # live ktiles: those whose k-range start <= last q of block
                kts = [kt for kt in range(len(segs)) if kt * P < gq + P]
                for i, kt in enumerate(kts):
                    o = max(kt * P - q0, 0)
                    col = segs[kt][0] + (j * P - o)
                    st = ets[:, h, col : col + P]
                    nc.tensor.matmul(
                        zp[:, h * H : (h + 1) * H], st, vA[:, kt, h, :],
                        start=(i == 0), stop=(i == len(kts) - 1))
                for i, kt in enumerate(kts):
                    o = max(kt * P - q0, 0)
                    col = segs[kt][0] + (j * P - o)
                    st = ets[:, h, col : col + P]
                    nc.tensor.matmul(
                        zp[:, 384 + h : 385 + h], st, ones,
                        start=(i == 0), stop=(i == len(kts) - 1))
            return zp

        def phase3(strip, j, zp, last):
            """normalize -> transpose -> out-proj -> store for q-tile j."""
            row0 = strip * QB + j * P
            r = smalls.tile([P, NHC], F32)
            nc.vector.reciprocal(r, zp[:, 384:390])
            zn = znp.tile([P, NHC, H], BF)
            rb = r.rearrange("p (h o) -> p h o", o=1).broadcast_to((P, NHC, H))
            nc.vector.tensor_mul(
                zn, zp[:, 0:384].rearrange("p (n h) -> p n h", n=NHC), rb)
            znt_ps = ps_m.tile([P, NPAIR, P], BF, tag="m")
            for t in range(NPAIR):
                nc.tensor.matmul(
                    znt_ps[:, t, :], zn[:, 2 * t : 2 * t + 2, :].rearrange(
                        "p n h -> p (n h)"),
                    ident, is_transpose=True)
            znt = zntp.tile([P, NPAIR, P], BF)
            nc.vector.tensor_copy(out=znt, in_=znt_ps)
            osb = outp.tile([P, D], BF)
            for dh in range(2):
                op = ps_m.tile([P, D // 2], F32, tag="m")
                for t in range(NPAIR):
                    nc.tensor.matmul(
                        op, znt[:, t, :],
                        wo[:, t, dh * (D // 2) : (dh + 1) * (D // 2)],
                        start=(t == 0), stop=(t == NPAIR - 1))
                sl = osb[:, dh * (D // 2) : (dh + 1) * (D // 2)]
                if last:
                    nc.scalar.copy(sl, op)
                else:
                    nc.vector.tensor_copy(out=sl, in_=op)
                nc.sync.dma_start(
                    out=out_d[row0 : row0 + P, dh * (D // 2) : (dh + 1) * (D // 2)],
                    in_=sl)

        # ================= schedule =================
        proj_qk(0, 0)
        scores_head(0, 0)
        scores_head(1, 0)
        scores_last_pair(0, 0)
        masks_head(0, 0)
        masks_head(1, 0)
        proj_qk(1, 0)
        scores_head(2, 0)
        scores_head(3, 0)
        scores_last_pair(1, 0)
        masks_head(2, 0)
        masks_head(3, 0)
        proj_qk(2, 0)
        scores_head(4, 0)
        scores_head(5, 0)
        scores_last_pair(2, 0)
        masks_head(4, 0)
        masks_head(5, 0)
        proj_qk(0, 1)
        scores_head(0, 1)
        scores_head(1, 1)
        scores_last_pair(0, 1)
        masks_head(0, 1)
        masks_head(1, 1)
        proj_qk(1, 1)
        for st in range(4):
            proj_v(st)
        zp0 = pv_qtile(0, 0)
        phase3(0, 0, zp0, False)
        scores_head(2, 1)
        scores_head(3, 1)
        scores_last_pair(1, 1)
        masks_head(2, 1)
        masks_head(3, 1)
        zp1 = pv_qtile(0, 1)
        phase3(0, 1, zp1, False)
        proj_qk(2, 1)
        scores_head(4, 1)
        scores_head(5, 1)
        scores_last_pair(2, 1)
        masks_head(4, 1)
        masks_head(5, 1)
        zp2 = pv_qtile(0, 2)
        phase3(0, 2, zp2, False)
        for st in range(4, NST):
            proj_v(st)
        zp3 = pv_qtile(0, 3)
        phase3(0, 3, zp3, False)
        zp4 = pv_qtile(1, 0)
        zp5 = pv_qtile(1, 1)
        phase3(1, 0, zp4, True)
        phase3(1, 1, zp5, True)
        zp6 = pv_qtile(1, 2)
        phase3(1, 2, zp6, True)
        zp7 = pv_qtile(1, 3)
        phase3(1, 3, zp7, True)

    if not nc.is_finalized():
        nc.finalize()
    return nc


def _get_program():
    if "nc" not in _CACHE:
        _CACHE["nc"] = _build()
    return _CACHE["nc"]


F8NP = ml_dtypes.float8_e4m3
BFNP = ml_dtypes.bfloat16


def _q8(a):
    return a.astype(F8NP)


def make_in_maps(normalized_resid_pre, W_Q, W_K, W_V, b_Q):
    x = np.asarray(normalized_resid_pre, np.float32)
    W_Q = np.asarray(W_Q, np.float32) * FW
    W_K = np.asarray(W_K, np.float32) * FW
    W_V = np.asarray(W_V, np.float32) * FW
    b_Q = np.asarray(b_Q, np.float32) * FW

    tri = np.triu(np.ones((P, P), np.float32)).astype(BFNP)
    ident = np.eye(P, dtype=np.float32).astype(BFNP)
    trid = np.concatenate([tri, ident], axis=1)

    in_maps = []
    for c in range(8):
        b, hg = divmod(c, 2)
        hs = slice(hg * NHC, (hg + 1) * NHC)
        xt = np.ascontiguousarray(x[b].T)          # [D, S]
        xh = _q8(xt)
        xhf = xh.astype(np.float32)
        xls = _q8((xt - xhf) * 16.0)
        xhd = (xhf / 16.0).astype(F8NP)            # exact exponent shift

        wall = np.zeros((D, WALL_W), F8NP)
        for wname, W in (("q", W_Q), ("k", W_K), ("v", W_V)):
            Wg = W[hs].transpose(1, 0, 2).reshape(D, HD)  # [D, 6*64]
            Wh = _q8(Wg)
            Whf = Wh.astype(np.float32)
            Whd = (Whf / 16.0).astype(F8NP)
            Wls = _q8((Wg - Whf) * 16.0)
            if wname == "v":
                c0 = WALL_COLS["vh"][0]
                wall[:, c0 : c0 + HD] = Wh
                c0 = WALL_COLS["vhd"][0]
                wall[:, c0 : c0 + HD] = Whd
                c0 = WALL_COLS["vls"][0]
                wall[:, c0 : c0 + HD] = Wls
            else:
                for g in range(NPAIR):
                    sl = slice(g * P, (g + 1) * P)
                    c0 = WALL_COLS[wname + "h" + str(g)][0]
                    wall[:, c0 : c0 + P] = Wh[:, sl]
                    c0 = WALL_COLS[wname + "hd" + str(g)][0]
                    wall[:, c0 : c0 + P] = Whd[:, sl]
                    if QK_3T:
                        c0 = WALL_COLS[wname + "ls" + str(g)][0]
                        wall[:, c0 : c0 + P] = Wls[:, sl]

        in_maps.append({
            "wall": wall,
            "xh": xh,
            "xls": xls,
            "xhd": xhd,
            "wo": None,  # filled in kernel()
            "bq": np.ascontiguousarray(b_Q[hs].reshape(NPAIR, P).T),
            "trid": trid,
        })
    return in_maps


def kernel(
    normalized_resid_pre, W_Q, W_K, W_V, W_O, b_Q, b_K, b_V, b_O, **_unused
):
    W_O = np.asarray(W_O, np.float32)
    b_V, b_O = np.asarray(b_V, np.float32), np.asarray(b_O, np.float32)
    in_maps = make_in_maps(normalized_resid_pre, W_Q, W_K, W_V, b_Q)
    for c in range(8):
        hg = c % 2
        hs = slice(hg * NHC, (hg + 1) * NHC)
        in_maps[c]["wo"] = np.ascontiguousarray(
            (W_O[hs].reshape(HD, D) / FW).astype(BFNP))

    nc = _get_program()
    res = run_bass_kernel_spmd(nc, in_maps, list(range(8))).results

    out = np.zeros((B, S, D), np.float32)
    for c in range(8):
        out[c // 2] += res[c]["out"].astype(np.float32)
    out += b_O + np.einsum("nh,nhd->d", b_V, W_O)
    return out


# revision 60
# speedup vs baseline: 1.0033x; 1.0033x over previous
"""Causal multi-head attention block on 8 NeuronCores (Trainium2, Bass/Tile).

Reference computation (per batch b):
  Q = x @ W_Q + b_Q ; K = x @ W_K (b_K dropped: softmax shift-invariant)
  scores = Q K^T / sqrt(H); causal mask; probs = softmax(scores)
  out = (probs @ V) @ W_O + b_O   (b_V folded on host: probs rows sum to 1)

Sharding: core c -> batch c//2, head-group c%2 (6 of 12 heads). Host sums
the two head-group partials per batch and adds b_O + sum_nh b_V[n,h]*W_O.

Numerics: weights are host-folded x32 so fp8(e4m3) residual splits stay in
the normal range. All projections are fp8 DoubleRow matmuls (0.5 cy/row,
two 128-contractions per instr): Q/K = xh*Wh + (16 xls)*(Wh/16) (2-term,
residual error ~ W quant, damped ~0.43x by the small score scale);
V = 3-term (adds (xh/16)*(16 Wls)) since V error passes straight through.
Q is stored as an fp8 hi+lo pair (bias folded into hi via tensor_scalar_add,
lo via scalar_tensor_tensor); K as fp8 hi. Scores: one DoubleRow instr per
(head, ktile): slots (Kh, Qhi) + (Kh dup via 0-stride stationary, Qlo).
exp runs on Activation into bf16 left-packed per-head ets strips, merged
across ktiles sharing a PSUM tile; the last kt-group of each head PAIR
shares one [P, 2, *] exp call (5 exp instrs per head total). PV is flipped:
stationary = ets [k, q-block] (full 128x128 array), moving = V bf16
[k, 64] -> z [q, h] in one PSUM bank per q-tile, with per-head denominator
columns via a ones-vector moving operand. Normalize = one reciprocal + one
0-stride-broadcast multiply per q-tile; z^T via bf16 PE transposes
(identity is_transpose matmuls); out-proj bf16; bf16 output (host widens
to f32). Causal masking multiplies the 8 diagonal ets blocks by a host
upper-tri bf16 matrix on GPSIMD.

Scheduling: single warm PE stream ordered so the Activation engine (the
exp bottleneck, ~33us busy) starts by ~7us and never starves: pair-0
projections -> strip-0 scores interleaved with remaining projections ->
strip-1 scores -> per-q-tile PV + phase3 (normalize/transpose/out-proj)
pipelined, with strip-1 out-copies on Activation after its last exp.
DMA priority: pair-0 Q/K weights, x hi, x lo, QK pair-1/2 weights,
x half-1, V weights, x/16 (V-only input), W_O. 24 fp8 DoubleRow warm-up
matmuls carry the PE p-state ramp through the DMA prologue.

CoreSim exec time: 48327 ns (baseline fp32r kernel: 81439 ns).
Relative error vs f32 reference: 1.37e-2 (gate: 2e-2).
"""

import sys

sys.path.insert(0, "/opt/trn_rl_repo")

from contextlib import ExitStack

import numpy as np
import ml_dtypes

import concourse.bass as bass
import concourse.tile as tile
from concourse import bacc, mybir
from concourse.bass_utils import run_bass_kernel_spmd

B, S, D, N, H = 4, 1024, 768, 12, 64
NHC = 6            # heads per core
NPAIR = NHC // 2   # head pairs (2 heads -> 128 partitions)
HD = NHC * H       # 384 packed head dim per core
P = 128
NDT = D // P       # 6 d-tiles
NST = S // P       # 8 k-tiles
QB = 512           # strip width (q)
FW = 32.0          # host weight fold (power of 2)
F32 = mybir.dt.float32
F8 = mybir.dt.float8e4
BF = mybir.dt.bfloat16
DR = mybir.MatmulPerfMode.DoubleRow
EXP_SCALE = 1.0 / (np.sqrt(float(H)) * FW * FW)

QK_3T = False      # 3-term QK projections (False: 2-term, faster/less exact)

# wall column layout (fp8 weights, priority-ordered for DMA chunking)
def _wall_layout():
    cols = {}
    off = 0
    def put(name, w):
        nonlocal off
        cols[name] = (off, w)
        off += w
    # chunk 0: pair0 Q then K (two priority sub-chunks)
    for t in ("qh", "qhd") + (("qls",) if QK_3T else ()):
        put(t + "0", P)
    for t in ("kh", "khd") + (("kls",) if QK_3T else ()):
        put(t + "0", P)
    c0 = off
    # chunk 1: V (all pairs)
    for t in ("vh", "vhd", "vls"):
        put(t, HD)
    c1 = off
    # chunks 2, 3: pairs 1, 2
    for g in (1, 2):
        for t in ("qh", "kh", "qhd", "khd") + (("qls", "kls") if QK_3T else ()):
            put(t + str(g), P)
    return cols, (0, c0, c1, (c1 + off) // 2, off)

WALL_COLS, WALL_CHUNKS = _wall_layout()
WALL_W = WALL_CHUNKS[-1]

# ets strip segment offsets (left-packed live columns per ktile)
def _segs(strip):
    offs, off = [], 0
    nkt = 4 if strip == 0 else NST
    for kt in range(nkt):
        if strip == 0:
            live = QB - kt * P
        else:
            live = min(S - kt * P, QB)
        offs.append((off, live))
        off += live
    return offs, off

SEG0, LEN0 = _segs(0)
SEG1, LEN1 = _segs(1)
# exp merge groups: lists of (ktile list, psum cols used)
EXPG0 = [[0, 1], [2, 3]]
EXPG1 = [[0, 1], [2, 3], [4, 5], [6, 7]]

_CACHE = {}


def _build():
    nc = bacc.Bacc()
    wall_d = nc.declare_dram_parameter("wall", [D, WALL_W], F8, isOutput=False)
    xh_d = nc.declare_dram_parameter("xh", [D, S], F8, isOutput=False)
    xls_d = nc.declare_dram_parameter("xls", [D, S], F8, isOutput=False)
    xhd_d = nc.declare_dram_parameter("xhd", [D, S], F8, isOutput=False)
    wo_d = nc.declare_dram_parameter("wo", [HD, D], BF, isOutput=False)
    bq_d = nc.declare_dram_parameter("bq", [P, NPAIR], F32, isOutput=False)
    trid_d = nc.declare_dram_parameter("trid", [P, 2 * P], BF, isOutput=False)
    out_d = nc.declare_dram_parameter("out", [S, D], BF, isOutput=True)

    wall_r = wall_d[:].rearrange("(t p) c -> p t c", p=P)
    xh_r = xh_d[:].rearrange("(t p) s -> p t s", p=P)
    xls_r = xls_d[:].rearrange("(t p) s -> p t s", p=P)
    xhd_r = xhd_d[:].rearrange("(t p) s -> p t s", p=P)
    wo_r = wo_d[:].rearrange("(t p) d -> p t d", p=P)

    with tile.TileContext(nc) as tc, ExitStack() as ctx:
        consts = ctx.enter_context(tc.tile_pool(name="consts", bufs=1))
        persist = ctx.enter_context(tc.tile_pool(name="persist", bufs=1))
        znp = ctx.enter_context(tc.tile_pool(name="znp", bufs=2))
        zntp = ctx.enter_context(tc.tile_pool(name="zntp", bufs=2))
        outp = ctx.enter_context(tc.tile_pool(name="outp", bufs=2))
        smalls = ctx.enter_context(tc.tile_pool(name="smalls", bufs=2))

        wall = consts.tile([P, NDT, WALL_W], F8)
        xh = consts.tile([P, NDT, S], F8)
        xls = consts.tile([P, NDT, S], F8)
        xhd = consts.tile([P, NDT, S], F8, name="xhd")
        wo = consts.tile([P, NPAIR, D], BF)
        bq = consts.tile([P, NPAIR], F32)
        trid = consts.tile([P, 2 * P], BF)
        tri = trid[:, 0:P]
        ident = trid[:, P : 2 * P]

        def wslice(name, g, t0, nt, c0, cw):
            """stationary slice [P, nt, cw] of wall tensor `name` (pair g)."""
            base = WALL_COLS[name + str(g) if name[0] in "qk" else name][0]
            return wall[:, t0 : t0 + nt, base + c0 : base + c0 + cw]

        # ---- DMA priority order (all on SP/sync queue) ----
        def dma_wall_chunk(i):
            c0, c1 = WALL_CHUNKS[i], WALL_CHUNKS[i + 1]
            nc.sync.dma_start(out=wall[:, :, c0:c1], in_=wall_r[:, :, c0:c1])

        def dma_x_half(sb_t, dram_r, s2):
            nc.sync.dma_start(
                out=sb_t[:, :, s2 * QB : (s2 + 1) * QB],
                in_=dram_r[:, :, s2 * QB : (s2 + 1) * QB],
            )

        dma_wall_chunk(0)
        nc.sync.dma_start(out=bq, in_=bq_d[:])
        dma_x_half(xh, xh_r, 0)
        nc.sync.dma_start(out=trid, in_=trid_d[:])
        dma_x_half(xls, xls_r, 0)
        dma_wall_chunk(2)      # pair 1
        dma_wall_chunk(3)      # pair 2
        dma_x_half(xh, xh_r, 1)
        dma_x_half(xls, xls_r, 1)
        dma_wall_chunk(1)      # V weights
        dma_x_half(xhd, xhd_r, 0)
        nc.sync.dma_start(out=wo, in_=wo_r)
        dma_x_half(xhd, xhd_r, 1)

        # ---- persistent activations ----
        qT8 = persist.tile([P, NPAIR, 2, S], F8)   # dim2: hi/lo
        kT8 = persist.tile([P, NPAIR, S], F8)
        vA = persist.tile([P, NST, NHC, H], BF)
        ets0 = persist.tile([P, NHC, LEN0], BF)
        ets1 = persist.tile([P, NHC, LEN1], BF)
        ones = persist.tile([P, 1], BF)
        nc.gpsimd.memset(ones, 1.0)

        ps_sm = ctx.enter_context(tc.tile_pool(name="ps_sm", bufs=2, space="PSUM"))
        ps_z = ctx.enter_context(tc.tile_pool(name="ps_z", bufs=2, space="PSUM"))
        ps_m = ctx.enter_context(tc.tile_pool(name="ps_m", bufs=2, space="PSUM"))

        # PE warm-up: carries the p-state ramp during the DMA prologue.
        dums = consts.tile([P, 2, QB], F8)
        nc.gpsimd.memset(dums, 0.0)
        # preload the Exp activation table off the critical path
        scr = smalls.tile([P, 1], BF, name="scr")
        nc.scalar.activation(scr, ones, mybir.ActivationFunctionType.Exp)
        wps = ps_m.tile([P, QB], F32, name="warm", tag="m")
        for i in range(24):
            nc.tensor.matmul(
                wps, dums[:, :, 0:P], dums, start=(i == 0), stop=(i == 23),
                perf_mode=DR,
            )

        def proj_qk(g, s2):
            """Q and K projections for pair g, s-half s2 (fp8 DR)."""
            s0 = s2 * QB
            k_on_act = (g == 0 and s2 == 0)
            for wname, ap_hi_lo in (("q", True), ("k", False)):
                pp = ps_m.tile([P, QB], F32, tag="m")
                ktp = [(t, t + 2) for t in (0, 2, 4)]
                for t, _ in ktp:
                    nc.tensor.matmul(
                        pp, wslice(wname + "h", g, t, 2, 0, P),
                        xh[:, t : t + 2, s0 : s0 + QB],
                        start=(t == 0), stop=False, perf_mode=DR)
                for i, (t, _) in enumerate(ktp):
                    nc.tensor.matmul(
                        pp, wslice(wname + "hd", g, t, 2, 0, P),
                        xls[:, t : t + 2, s0 : s0 + QB],
                        start=False, stop=(not QK_3T and i == 2),
                        perf_mode=DR)
                if QK_3T:
                    for i, (t, _) in enumerate(ktp):
                        nc.tensor.matmul(
                            pp, wslice(wname + "ls", g, t, 2, 0, P),
                            xhd[:, t : t + 2, s0 : s0 + QB],
                            start=False, stop=(i == 2), perf_mode=DR)
                if ap_hi_lo:
                    nc.vector.tensor_scalar_add(
                        qT8[:, g, 0, s0 : s0 + QB], pp, bq[:, g : g + 1])
                    nc.vector.scalar_tensor_tensor(
                        out=qT8[:, g, 1, s0 : s0 + QB], in0=pp,
                        scalar=bq[:, g : g + 1], in1=qT8[:, g, 0, s0 : s0 + QB],
                        op0=mybir.AluOpType.add, op1=mybir.AluOpType.subtract)
                elif k_on_act:
                    nc.scalar.copy(kT8[:, g, s0 : s0 + QB], pp)
                else:
                    nc.vector.tensor_copy(out=kT8[:, g, s0 : s0 + QB], in_=pp)

        def proj_v(st):
            """V projection for s-tile st -> vA bf16 (fp8 DR, 3-term)."""
            pp = ps_m.tile([P, HD], F32, tag="m")
            for t in (0, 2, 4):
                nc.tensor.matmul(
                    pp, xh[:, t : t + 2, st * P : (st + 1) * P],
                    wslice("vh", 0, t, 2, 0, HD),
                    start=(t == 0), stop=False, perf_mode=DR)
            for i, t in enumerate((0, 2, 4)):
                nc.tensor.matmul(
                    pp, xls[:, t : t + 2, st * P : (st + 1) * P],
                    wslice("vhd", 0, t, 2, 0, HD),
                    start=False, stop=False, perf_mode=DR)
            for i, t in enumerate((0, 2, 4)):
                nc.tensor.matmul(
                    pp, xhd[:, t : t + 2, st * P : (st + 1) * P],
                    wslice("vls", 0, t, 2, 0, HD),
                    start=False, stop=(i == 2), perf_mode=DR)
            nc.vector.tensor_copy(
                out=vA[:, st, :, :], in_=pp.rearrange("p (n h) -> p n h", n=NHC))

        def score_mm(sm_slice, h, kt, strip):
            """One score DR for (head, ktile) into the given psum slice."""
            g, e = divmod(h, 2)
            hp = e * H
            segs = SEG0 if strip == 0 else SEG1
            q0 = strip * QB
            live = segs[kt][1]
            o = QB - live
            kst = kT8[hp : hp + H, g, kt * P : (kt + 1) * P]
            kst = kst.rearrange("p (o m) -> p o m", o=1).broadcast_to((H, 2, P))
            nc.tensor.matmul(
                sm_slice, kst,
                qT8[hp : hp + H, g, :, q0 + o : q0 + QB],
                start=True, stop=True, perf_mode=DR)

        def scores_head(h, strip):
            """Merged-group score DRs + exps for head h (all but last group)."""
            ets = ets0 if strip == 0 else ets1
            segs = SEG0 if strip == 0 else SEG1
            groups = (EXPG0 if strip == 0 else EXPG1)[:-1]
            for kts in groups:
                used = sum(segs[kt][1] for kt in kts)
                sm = ps_sm.tile([P, 2 * QB], F32, tag="sm")
                pcol = 0
                for kt in kts:
                    live = segs[kt][1]
                    score_mm(sm[:, pcol : pcol + live], h, kt, strip)
                    pcol += live
                seg0 = segs[kts[0]][0]
                nc.scalar.activation(
                    ets[:, h, seg0 : seg0 + used], sm[:, 0:used],
                    mybir.ActivationFunctionType.Exp, scale=EXP_SCALE)

        def scores_last_pair(g, strip):
            """Last kt-group of both heads of pair g: one shared exp."""
            ets = ets0 if strip == 0 else ets1
            segs = SEG0 if strip == 0 else SEG1
            kts = (EXPG0 if strip == 0 else EXPG1)[-1]
            used = sum(segs[kt][1] for kt in kts)
            sm = ps_sm.tile([P, 2, QB], F32, tag="sm")
            for i in range(2):
                pcol = 0
                for kt in kts:
                    live = segs[kt][1]
                    score_mm(sm[:, i, pcol : pcol + live], 2 * g + i, kt, strip)
                    pcol += live
            seg0 = segs[kts[0]][0]
            nc.scalar.activation(
                ets[:, 2 * g : 2 * g + 2, seg0 : seg0 + used], sm[:, :, 0:used],
                mybir.ActivationFunctionType.Exp, scale=EXP_SCALE)

        def masks_head(h, strip):
            ets = ets0 if strip == 0 else ets1
            segs = SEG0 if strip == 0 else SEG1
            dkts = range(4) if strip == 0 else range(4, 8)
            for kt in dkts:
                so = segs[kt][0]
                nc.gpsimd.tensor_mul(
                    ets[:, h, so : so + P], ets[:, h, so : so + P], tri)

        def pv_qtile(strip, j):
            """z psum for q-tile j of strip: all heads' PV + denominators."""
            ets = ets0 if strip == 0 else ets1
            segs = SEG0 if strip == 0 else SEG1
            q0 = strip * QB
            zp = ps_z.tile([P, 390], F32, tag="z")
            gq = q0 + j * P  # global q block start
            for h in range(NHC):
                kts = [kt for kt in range(len(segs))
                       if kt * P <= gq + P - 1 and True]
                # live ktiles: those whose k-range start <= last q of block
                kts = [kt for kt in range(len(segs)) if kt * P < gq + P]
                for i, kt in enumerate(kts):
                    o = max(kt * P - q0, 0)
                    col = segs[kt][0] + (j * P - o)
                    st = ets[:, h, col : col + P]
                    nc.tensor.matmul(
                        zp[:, h * H : (h + 1) * H], st, vA[:, kt, h, :],
                        start=(i == 0), stop=(i == len(kts) - 1))
                for i, kt in enumerate(kts):
                    o = max(kt * P - q0, 0)
                    col = segs[kt][0] + (j * P - o)
                    st = ets[:, h, col : col + P]
                    nc.tensor.matmul(
                        zp[:, 384 + h : 385 + h], st, ones,
                        start=(i == 0), stop=(i == len(kts) - 1))
            return zp

        def phase3(strip, j, zp, last):
            """normalize -> transpose -> out-proj -> store for q-tile j."""
            row0 = strip * QB + j * P
            r = smalls.tile([P, NHC], F32)
            nc.vector.reciprocal(r, zp[:, 384:390])
            zn = znp.tile([P, NHC, H], BF)
            rb = r.rearrange("p (h o) -> p h o", o=1).broadcast_to((P, NHC, H))
            nc.vector.tensor_mul(
                zn, zp[:, 0:384].rearrange("p (n h) -> p n h", n=NHC), rb)
            znt_ps = ps_m.tile([P, NPAIR, P], BF, tag="m")
            for t in range(NPAIR):
                nc.tensor.matmul(
                    znt_ps[:, t, :], zn[:, 2 * t : 2 * t + 2, :].rearrange(
                        "p n h -> p (n h)"),
                    ident, is_transpose=True)
            znt = zntp.tile([P, NPAIR, P], BF)
            nc.vector.tensor_copy(out=znt, in_=znt_ps)
            osb = outp.tile([P, D], BF)
            for dh in range(2):
                op = ps_m.tile([P, D // 2], F32, tag="m")
                for t in range(NPAIR):
                    nc.tensor.matmul(
                        op, znt[:, t, :],
                        wo[:, t, dh * (D // 2) : (dh + 1) * (D // 2)],
                        start=(t == 0), stop=(t == NPAIR - 1))
                sl = osb[:, dh * (D // 2) : (dh + 1) * (D // 2)]
                if last:
                    nc.scalar.copy(sl, op)
                else:
                    nc.vector.tensor_copy(out=sl, in_=op)
                nc.sync.dma_start(
                    out=out_d[row0 : row0 + P, dh * (D // 2) : (dh + 1) * (D // 2)],
                    in_=sl)

        # ================= schedule =================
        proj_qk(0, 0)
        scores_head(0, 0)
        scores_head(1, 0)
        scores_last_pair(0, 0)
        masks_head(0, 0)
        masks_head(1, 0)
        proj_qk(1, 0)
        scores_head(2, 0)
        scores_head(3, 0)
        scores_last_pair(1, 0)
        masks_head(2, 0)
        masks_head(3, 0)
        proj_qk(2, 0)
        scores_head(4, 0)
        scores_head(5, 0)
        scores_last_pair(2, 0)
        masks_head(4, 0)
        masks_head(5, 0)
        proj_qk(0, 1)
        scores_head(0, 1)
        scores_head(1, 1)
        scores_last_pair(0, 1)
        masks_head(0, 1)
        masks_head(1, 1)
        proj_qk(1, 1)
        for st in range(4):
            proj_v(st)
        zp0 = pv_qtile(0, 0)
        phase3(0, 0, zp0, False)
        scores_head(2, 1)
        scores_head(3, 1)
        scores_last_pair(1, 1)
        masks_head(2, 1)
        masks_head(3, 1)
        zp1 = pv_qtile(0, 1)
        phase3(0, 1, zp1, False)
        proj_qk(2, 1)
        scores_head(4, 1)
        scores_head(5, 1)
        scores_last_pair(2, 1)
        masks_head(4, 1)
        masks_head(5, 1)
        zp2 = pv_qtile(0, 2)
        phase3(0, 2, zp2, False)
        for st in range(4, NST):
            proj_v(st)
        zp3 = pv_qtile(0, 3)
        phase3(0, 3, zp3, False)
        zp4 = pv_qtile(1, 0)
        zp5 = pv_qtile(1, 1)
        phase3(1, 0, zp4, True)
        phase3(1, 1, zp5, True)
        zp6 = pv_qtile(1, 2)
        phase3(1, 2, zp6, True)
        zp7 = pv_qtile(1, 3)
        phase3(1, 3, zp7, True)

    if not nc.is_finalized():
        nc.finalize()
    return nc


def _get_program():
    if "nc" not in _CACHE:
        _CACHE["nc"] = _build()
    return _CACHE["nc"]


F8NP = ml_dtypes.float8_e4m3
BFNP = ml_dtypes.bfloat16


def _q8(a):
    return a.astype(F8NP)


def make_in_maps(normalized_resid_pre, W_Q, W_K, W_V, b_Q):
    x = np.asarray(normalized_resid_pre, np.float32)
    W_Q = np.asarray(W_Q, np.float32) * FW
    W_K = np.asarray(W_K, np.float32) * FW
    W_V = np.asarray(W_V, np.float32) * FW
    b_Q = np.asarray(b_Q, np.float32) * FW

    tri = np.triu(np.ones((P, P), np.float32)).astype(BFNP)
    ident = np.eye(P, dtype=np.float32).astype(BFNP)
    trid = np.concatenate([tri, ident], axis=1)

    in_maps = []
    for c in range(8):
        b, hg = divmod(c, 2)
        hs = slice(hg * NHC, (hg + 1) * NHC)
        xt = np.ascontiguousarray(x[b].T)          # [D, S]
        xh = _q8(xt)
        xhf = xh.astype(np.float32)
        xls = _q8((xt - xhf) * 16.0)
        xhd = (xhf / 16.0).astype(F8NP)            # exact exponent shift

        wall = np.zeros((D, WALL_W), F8NP)
        for wname, W in (("q", W_Q), ("k", W_K), ("v", W_V)):
            Wg = W[hs].transpose(1, 0, 2).reshape(D, HD)  # [D, 6*64]
            Wh = _q8(Wg)
            Whf = Wh.astype(np.float32)
            Whd = (Whf / 16.0).astype(F8NP)
            Wls = _q8((Wg - Whf) * 16.0)
            if wname == "v":
                c0 = WALL_COLS["vh"][0]
                wall[:, c0 : c0 + HD] = Wh
                c0 = WALL_COLS["vhd"][0]
                wall[:, c0 : c0 + HD] = Whd
                c0 = WALL_COLS["vls"][0]
                wall[:, c0 : c0 + HD] = Wls
            else:
                for g in range(NPAIR):
                    sl = slice(g * P, (g + 1) * P)
                    c0 = WALL_COLS[wname + "h" + str(g)][0]
                    wall[:, c0 : c0 + P] = Wh[:, sl]
                    c0 = WALL_COLS[wname + "hd" + str(g)][0]
                    wall[:, c0 : c0 + P] = Whd[:, sl]
                    if QK_3T:
                        c0 = WALL_COLS[wname + "ls" + str(g)][0]
                        wall[:, c0 : c0 + P] = Wls[:, sl]

        in_maps.append({
            "wall": wall,
            "xh": xh,
            "xls": xls,
            "xhd": xhd,
            "wo": None,  # filled in kernel()
            "bq": np.ascontiguousarray(b_Q[hs].reshape(NPAIR, P).T),
            "trid": trid,
        })
    return in_maps


def kernel(
    normalized_resid_pre, W_Q, W_K, W_V, W_O, b_Q, b_K, b_V, b_O, **_unused
):
    W_O = np.asarray(W_O, np.float32)
    b_V, b_O = np.asarray(b_V, np.float32), np.asarray(b_O, np.float32)
    in_maps = make_in_maps(normalized_resid_pre, W_Q, W_K, W_V, b_Q)
    for c in range(8):
        hg = c % 2
        hs = slice(hg * NHC, (hg + 1) * NHC)
        in_maps[c]["wo"] = np.ascontiguousarray(
            (W_O[hs].reshape(HD, D) / FW).astype(BFNP))

    nc = _get_program()
    res = run_bass_kernel_spmd(nc, in_maps, list(range(8))).results

    out = np.zeros((B, S, D), np.float32)
    for c in range(8):
        out[c // 2] += res[c]["out"].astype(np.float32)
    out += b_O + np.einsum("nh,nhd->d", b_V, W_O)
    return out


# revision 61
# speedup vs baseline: 1.0049x; 1.0016x over previous
"""Causal multi-head attention block on 8 NeuronCores (Trainium2, Bass/Tile).

Reference computation (per batch b):
  Q = x @ W_Q + b_Q ; K = x @ W_K (b_K dropped: softmax shift-invariant)
  scores = Q K^T / sqrt(H); causal mask; probs = softmax(scores)
  out = (probs @ V) @ W_O + b_O   (b_V folded on host: probs rows sum to 1)

Sharding: core c -> batch c//2, head-group c%2 (6 of 12 heads). Host sums
the two head-group partials per batch and adds b_O + sum_nh b_V[n,h]*W_O.

Numerics: weights are host-folded x32 so fp8(e4m3) residual splits stay in
the normal range. All projections are fp8 DoubleRow matmuls (0.5 cy/row,
two 128-contractions per instr): Q/K = xh*Wh + (16 xls)*(Wh/16) (2-term,
residual error ~ W quant, damped ~0.43x by the small score scale);
V = 3-term (adds (xh/16)*(16 Wls)) since V error passes straight through.
Q is stored as an fp8 hi+lo pair (bias folded into hi via tensor_scalar_add,
lo via scalar_tensor_tensor); K as fp8 hi. Scores: one DoubleRow instr per
(head, ktile): slots (Kh, Qhi) + (Kh dup via 0-stride stationary, Qlo).
exp runs on Activation into bf16 left-packed per-head ets strips, merged
across ktiles sharing a PSUM tile; the last kt-group of each head PAIR
shares one [P, 2, *] exp call (5 exp instrs per head total). PV is flipped:
stationary = ets [k, q-block] (full 128x128 array), moving = V bf16
[k, 64] -> z [q, h] in one PSUM bank per q-tile, with per-head denominator
columns via a ones-vector moving operand. Normalize = one reciprocal + one
0-stride-broadcast multiply per q-tile; z^T via bf16 PE transposes
(identity is_transpose matmuls); out-proj bf16; bf16 output (host widens
to f32). Causal masking multiplies the 8 diagonal ets blocks by a host
upper-tri bf16 matrix on GPSIMD.

Scheduling: single warm PE stream ordered so the Activation engine (the
exp bottleneck, ~33us busy) starts by ~7us and never starves: pair-0
projections -> strip-0 scores interleaved with remaining projections ->
strip-1 scores -> per-q-tile PV + phase3 (normalize/transpose/out-proj)
pipelined, with strip-1 out-copies on Activation after its last exp.
DMA priority: pair-0 Q/K weights, x hi, x lo, QK pair-1/2 weights,
x half-1, V weights, x/16 (V-only input), W_O. 24 fp8 DoubleRow warm-up
matmuls carry the PE p-state ramp through the DMA prologue.

CoreSim exec time: 48327 ns (baseline fp32r kernel: 81439 ns).
Relative error vs f32 reference: 1.37e-2 (gate: 2e-2).
"""

import sys

sys.path.insert(0, "/opt/trn_rl_repo")

from contextlib import ExitStack

import numpy as np
import ml_dtypes

import concourse.bass as bass
import concourse.tile as tile
from concourse import bacc, mybir
from concourse.bass_utils import run_bass_kernel_spmd

B, S, D, N, H = 4, 1024, 768, 12, 64
NHC = 6            # heads per core
NPAIR = NHC // 2   # head pairs (2 heads -> 128 partitions)
HD = NHC * H       # 384 packed head dim per core
P = 128
NDT = D // P       # 6 d-tiles
NST = S // P       # 8 k-tiles
QB = 512           # strip width (q)
FW = 32.0          # host weight fold (power of 2)
F32 = mybir.dt.float32
F8 = mybir.dt.float8e4
BF = mybir.dt.bfloat16
DR = mybir.MatmulPerfMode.DoubleRow
EXP_SCALE = 1.0 / (np.sqrt(float(H)) * FW * FW)

QK_3T = False      # 3-term QK projections (False: 2-term, faster/less exact)

# wall column layout (fp8 weights, priority-ordered for DMA chunking)
def _wall_layout():
    cols = {}
    off = 0
    def put(name, w):
        nonlocal off
        cols[name] = (off, w)
        off += w
    # chunk 0: pair0 Q then K (two priority sub-chunks)
    for t in ("qh", "qhd") + (("qls",) if QK_3T else ()):
        put(t + "0", P)
    for t in ("kh", "khd") + (("kls",) if QK_3T else ()):
        put(t + "0", P)
    c0 = off
    # chunk 1: V (all pairs)
    for t in ("vh", "vhd", "vls"):
        put(t, HD)
    c1 = off
    # chunks 2, 3: pairs 1, 2
    for g in (1, 2):
        for t in ("qh", "kh", "qhd", "khd") + (("qls", "kls") if QK_3T else ()):
            put(t + str(g), P)
    return cols, (0, c0, c1, (c1 + off) // 2, off)

WALL_COLS, WALL_CHUNKS = _wall_layout()
WALL_W = WALL_CHUNKS[-1]

# ets strip segment offsets (left-packed live columns per ktile)
def _segs(strip):
    offs, off = [], 0
    nkt = 4 if strip == 0 else NST
    for kt in range(nkt):
        if strip == 0:
            live = QB - kt * P
        else:
            live = min(S - kt * P, QB)
        offs.append((off, live))
        off += live
    return offs, off

SEG0, LEN0 = _segs(0)
SEG1, LEN1 = _segs(1)
# exp merge groups: lists of (ktile list, psum cols used)
EXPG0 = [[0, 1], [2, 3]]
EXPG1 = [[0, 1], [2, 3], [4, 5], [6, 7]]

_CACHE = {}


def _build():
    nc = bacc.Bacc()
    wall_d = nc.declare_dram_parameter("wall", [D, WALL_W], F8, isOutput=False)
    xh_d = nc.declare_dram_parameter("xh", [D, S], F8, isOutput=False)
    xls_d = nc.declare_dram_parameter("xls", [D, S], F8, isOutput=False)
    xhd_d = nc.declare_dram_parameter("xhd", [D, S], F8, isOutput=False)
    wo_d = nc.declare_dram_parameter("wo", [HD, D], BF, isOutput=False)
    bq_d = nc.declare_dram_parameter("bq", [P, NPAIR], F32, isOutput=False)
    trid_d = nc.declare_dram_parameter("trid", [P, 2 * P], BF, isOutput=False)
    out_d = nc.declare_dram_parameter("out", [S, D], BF, isOutput=True)

    wall_r = wall_d[:].rearrange("(t p) c -> p t c", p=P)
    xh_r = xh_d[:].rearrange("(t p) s -> p t s", p=P)
    xls_r = xls_d[:].rearrange("(t p) s -> p t s", p=P)
    xhd_r = xhd_d[:].rearrange("(t p) s -> p t s", p=P)
    wo_r = wo_d[:].rearrange("(t p) d -> p t d", p=P)

    with tile.TileContext(nc) as tc, ExitStack() as ctx:
        consts = ctx.enter_context(tc.tile_pool(name="consts", bufs=1))
        persist = ctx.enter_context(tc.tile_pool(name="persist", bufs=1))
        znp = ctx.enter_context(tc.tile_pool(name="znp", bufs=2))
        zntp = ctx.enter_context(tc.tile_pool(name="zntp", bufs=2))
        outp = ctx.enter_context(tc.tile_pool(name="outp", bufs=2))
        smalls = ctx.enter_context(tc.tile_pool(name="smalls", bufs=2))

        wall = consts.tile([P, NDT, WALL_W], F8)
        xh = consts.tile([P, NDT, S], F8)
        xls = consts.tile([P, NDT, S], F8)
        xhd = consts.tile([P, NDT, S], F8, name="xhd")
        wo = consts.tile([P, NPAIR, D], BF)
        bq = consts.tile([P, NPAIR], F32)
        trid = consts.tile([P, 2 * P], BF)
        tri = trid[:, 0:P]
        ident = trid[:, P : 2 * P]

        def wslice(name, g, t0, nt, c0, cw):
            """stationary slice [P, nt, cw] of wall tensor `name` (pair g)."""
            base = WALL_COLS[name + str(g) if name[0] in "qk" else name][0]
            return wall[:, t0 : t0 + nt, base + c0 : base + c0 + cw]

        # ---- DMA priority order (all on SP/sync queue) ----
        def dma_wall_chunk(i):
            c0, c1 = WALL_CHUNKS[i], WALL_CHUNKS[i + 1]
            nc.sync.dma_start(out=wall[:, :, c0:c1], in_=wall_r[:, :, c0:c1])

        def dma_x_half(sb_t, dram_r, s2):
            nc.sync.dma_start(
                out=sb_t[:, :, s2 * QB : (s2 + 1) * QB],
                in_=dram_r[:, :, s2 * QB : (s2 + 1) * QB],
            )

        dma_wall_chunk(0)
        nc.sync.dma_start(out=bq, in_=bq_d[:])
        dma_x_half(xh, xh_r, 0)
        nc.sync.dma_start(out=trid, in_=trid_d[:])
        dma_x_half(xls, xls_r, 0)
        dma_wall_chunk(2)      # pair 1
        dma_wall_chunk(3)      # pair 2
        dma_x_half(xh, xh_r, 1)
        dma_x_half(xls, xls_r, 1)
        dma_wall_chunk(1)      # V weights
        dma_x_half(xhd, xhd_r, 0)
        nc.sync.dma_start(out=wo, in_=wo_r)
        dma_x_half(xhd, xhd_r, 1)

        # ---- persistent activations ----
        qT8 = persist.tile([P, NPAIR, 2, S], F8)   # dim2: hi/lo
        kT8 = persist.tile([P, NPAIR, S], F8)
        vA = persist.tile([P, NST, NHC, H], BF)
        ets0 = persist.tile([P, NHC, LEN0], BF)
        ets1 = persist.tile([P, NHC, LEN1], BF)
        ones = persist.tile([P, 1], BF)
        nc.gpsimd.memset(ones, 1.0)

        ps_sm = ctx.enter_context(tc.tile_pool(name="ps_sm", bufs=2, space="PSUM"))
        ps_z = ctx.enter_context(tc.tile_pool(name="ps_z", bufs=2, space="PSUM"))
        ps_m = ctx.enter_context(tc.tile_pool(name="ps_m", bufs=2, space="PSUM"))

        # PE warm-up: carries the p-state ramp during the DMA prologue.
        dums = consts.tile([P, 2, QB], F8)
        nc.gpsimd.memset(dums, 0.0)
        # preload the Exp activation table off the critical path
        scr = smalls.tile([P, 1], BF, name="scr")
        nc.scalar.activation(scr, ones, mybir.ActivationFunctionType.Exp)
        wps = ps_m.tile([P, QB], F32, name="warm", tag="m")
        for i in range(24):
            nc.tensor.matmul(
                wps, dums[:, :, 0:P], dums, start=(i == 0), stop=(i == 23),
                perf_mode=DR,
            )

        def proj_qk(g, s2):
            """Q and K projections for pair g, s-half s2 (fp8 DR)."""
            s0 = s2 * QB
            k_on_act = (g == 0 and s2 == 0)
            for wname, ap_hi_lo in (("q", True), ("k", False)):
                pp = ps_m.tile([P, QB], F32, tag="m")
                ktp = [(t, t + 2) for t in (0, 2, 4)]
                for t, _ in ktp:
                    nc.tensor.matmul(
                        pp, wslice(wname + "h", g, t, 2, 0, P),
                        xh[:, t : t + 2, s0 : s0 + QB],
                        start=(t == 0), stop=False, perf_mode=DR)
                for i, (t, _) in enumerate(ktp):
                    nc.tensor.matmul(
                        pp, wslice(wname + "hd", g, t, 2, 0, P),
                        xls[:, t : t + 2, s0 : s0 + QB],
                        start=False, stop=(not QK_3T and i == 2),
                        perf_mode=DR)
                if QK_3T:
                    for i, (t, _) in enumerate(ktp):
                        nc.tensor.matmul(
                            pp, wslice(wname + "ls", g, t, 2, 0, P),
                            xhd[:, t : t + 2, s0 : s0 + QB],
                            start=False, stop=(i == 2), perf_mode=DR)
                if ap_hi_lo:
                    nc.vector.tensor_scalar_add(
                        qT8[:, g, 0, s0 : s0 + QB], pp, bq[:, g : g + 1])
                    nc.vector.scalar_tensor_tensor(
                        out=qT8[:, g, 1, s0 : s0 + QB], in0=pp,
                        scalar=bq[:, g : g + 1], in1=qT8[:, g, 0, s0 : s0 + QB],
                        op0=mybir.AluOpType.add, op1=mybir.AluOpType.subtract)
                elif k_on_act:
                    nc.scalar.copy(kT8[:, g, s0 : s0 + QB], pp)
                else:
                    nc.vector.tensor_copy(out=kT8[:, g, s0 : s0 + QB], in_=pp)

        def proj_v(st):
            """V projection for s-tile st -> vA bf16 (fp8 DR, 3-term)."""
            pp = ps_m.tile([P, HD], F32, tag="m")
            for t in (0, 2, 4):
                nc.tensor.matmul(
                    pp, xh[:, t : t + 2, st * P : (st + 1) * P],
                    wslice("vh", 0, t, 2, 0, HD),
                    start=(t == 0), stop=False, perf_mode=DR)
            for i, t in enumerate((0, 2, 4)):
                nc.tensor.matmul(
                    pp, xls[:, t : t + 2, st * P : (st + 1) * P],
                    wslice("vhd", 0, t, 2, 0, HD),
                    start=False, stop=False, perf_mode=DR)
            for i, t in enumerate((0, 2, 4)):
                nc.tensor.matmul(
                    pp, xhd[:, t : t + 2, st * P : (st + 1) * P],
                    wslice("vls", 0, t, 2, 0, HD),
                    start=False, stop=(i == 2), perf_mode=DR)
            nc.vector.tensor_copy(
                out=vA[:, st, :, :], in_=pp.rearrange("p (n h) -> p n h", n=NHC))

        def score_mm(sm_slice, h, kt, strip):
            """One score DR for (head, ktile) into the given psum slice."""
            g, e = divmod(h, 2)
            hp = e * H
            segs = SEG0 if strip == 0 else SEG1
            q0 = strip * QB
            live = segs[kt][1]
            o = QB - live
            kst = kT8[hp : hp + H, g, kt * P : (kt + 1) * P]
            kst = kst.rearrange("p (o m) -> p o m", o=1).broadcast_to((H, 2, P))
            nc.tensor.matmul(
                sm_slice, kst,
                qT8[hp : hp + H, g, :, q0 + o : q0 + QB],
                start=True, stop=True, perf_mode=DR)

        def scores_head(h, strip):
            """Merged-group score DRs + exps for head h (all but last group)."""
            ets = ets0 if strip == 0 else ets1
            segs = SEG0 if strip == 0 else SEG1
            groups = (EXPG0 if strip == 0 else EXPG1)[:-1]
            for kts in groups:
                used = sum(segs[kt][1] for kt in kts)
                sm = ps_sm.tile([P, 2 * QB], F32, tag="sm")
                pcol = 0
                for kt in kts:
                    live = segs[kt][1]
                    score_mm(sm[:, pcol : pcol + live], h, kt, strip)
                    pcol += live
                seg0 = segs[kts[0]][0]
                nc.scalar.activation(
                    ets[:, h, seg0 : seg0 + used], sm[:, 0:used],
                    mybir.ActivationFunctionType.Exp, scale=EXP_SCALE)

        def scores_last_pair(g, strip):
            """Last kt-group of both heads of pair g: one shared exp."""
            ets = ets0 if strip == 0 else ets1
            segs = SEG0 if strip == 0 else SEG1
            kts = (EXPG0 if strip == 0 else EXPG1)[-1]
            used = sum(segs[kt][1] for kt in kts)
            sm = ps_sm.tile([P, 2, QB], F32, tag="sm")
            for i in range(2):
                pcol = 0
                for kt in kts:
                    live = segs[kt][1]
                    score_mm(sm[:, i, pcol : pcol + live], 2 * g + i, kt, strip)
                    pcol += live
            seg0 = segs[kts[0]][0]
            nc.scalar.activation(
                ets[:, 2 * g : 2 * g + 2, seg0 : seg0 + used], sm[:, :, 0:used],
                mybir.ActivationFunctionType.Exp, scale=EXP_SCALE)

        def masks_head(h, strip):
            ets = ets0 if strip == 0 else ets1
            segs = SEG0 if strip == 0 else SEG1
            dkts = range(4) if strip == 0 else range(4, 8)
            for kt in dkts:
                so = segs[kt][0]
                nc.gpsimd.tensor_mul(
                    ets[:, h, so : so + P], ets[:, h, so : so + P], tri)

        def pv_qtile(strip, j):
            """z psum for q-tile j of strip: all heads' PV + denominators."""
            ets = ets0 if strip == 0 else ets1
            segs = SEG0 if strip == 0 else SEG1
            q0 = strip * QB
            zp = ps_z.tile([P, 390], F32, tag="z")
            gq = q0 + j * P  # global q block start
            for h in range(NHC):
                kts = [kt for kt in range(len(segs))
                       if kt * P <= gq + P - 1 and True]
                # live ktiles: those whose k-range start <= last q of block
                kts = [kt for kt in range(len(segs)) if kt * P < gq + P]
                for i, kt in enumerate(kts):
                    o = max(kt * P - q0, 0)
                    col = segs[kt][0] + (j * P - o)
                    st = ets[:, h, col : col + P]
                    nc.tensor.matmul(
                        zp[:, h * H : (h + 1) * H], st, vA[:, kt, h, :],
                        start=(i == 0), stop=(i == len(kts) - 1))
                for i, kt in enumerate(kts):
                    o = max(kt * P - q0, 0)
                    col = segs[kt][0] + (j * P - o)
                    st = ets[:, h, col : col + P]
                    nc.tensor.matmul(
                        zp[:, 384 + h : 385 + h], st, ones,
                        start=(i == 0), stop=(i == len(kts) - 1))
            return zp

        def phase3(strip, j, zp, last):
            """normalize -> transpose -> out-proj -> store for q-tile j."""
            row0 = strip * QB + j * P
            r = smalls.tile([P, NHC], F32)
            nc.vector.reciprocal(r, zp[:, 384:390])
            zn = znp.tile([P, NHC, H], BF)
            rb = r.rearrange("p (h o) -> p h o", o=1).broadcast_to((P, NHC, H))
            nc.vector.tensor_mul(
                zn, zp[:, 0:384].rearrange("p (n h) -> p n h", n=NHC), rb)
            zpool = ps_sm if strip == 1 else ps_m
            znt_ps = zpool.tile([P, NPAIR, P], BF, tag="sm" if strip == 1 else "m")
            for t in range(NPAIR):
                nc.tensor.matmul(
                    znt_ps[:, t, :], zn[:, 2 * t : 2 * t + 2, :].rearrange(
                        "p n h -> p (n h)"),
                    ident, is_transpose=True)
            znt = zntp.tile([P, NPAIR, P], BF)
            nc.vector.tensor_copy(out=znt, in_=znt_ps)
            osb = outp.tile([P, D], BF)
            for dh in range(2):
                op = ps_m.tile([P, D // 2], F32, tag="m")
                for t in range(NPAIR):
                    nc.tensor.matmul(
                        op, znt[:, t, :],
                        wo[:, t, dh * (D // 2) : (dh + 1) * (D // 2)],
                        start=(t == 0), stop=(t == NPAIR - 1))
                sl = osb[:, dh * (D // 2) : (dh + 1) * (D // 2)]
                if last:
                    nc.scalar.copy(sl, op)
                else:
                    nc.vector.tensor_copy(out=sl, in_=op)
                nc.sync.dma_start(
                    out=out_d[row0 : row0 + P, dh * (D // 2) : (dh + 1) * (D // 2)],
                    in_=sl)

        # ================= schedule =================
        proj_qk(0, 0)
        scores_head(0, 0)
        scores_head(1, 0)
        scores_last_pair(0, 0)
        masks_head(0, 0)
        masks_head(1, 0)
        proj_qk(1, 0)
        scores_head(2, 0)
        scores_head(3, 0)
        scores_last_pair(1, 0)
        masks_head(2, 0)
        masks_head(3, 0)
        proj_qk(2, 0)
        scores_head(4, 0)
        scores_head(5, 0)
        scores_last_pair(2, 0)
        masks_head(4, 0)
        masks_head(5, 0)
        proj_qk(0, 1)
        scores_head(0, 1)
        scores_head(1, 1)
        scores_last_pair(0, 1)
        masks_head(0, 1)
        masks_head(1, 1)
        proj_qk(1, 1)
        for st in range(4):
            proj_v(st)
        zp0 = pv_qtile(0, 0)
        phase3(0, 0, zp0, False)
        scores_head(2, 1)
        scores_head(3, 1)
        scores_last_pair(1, 1)
        masks_head(2, 1)
        masks_head(3, 1)
        zp1 = pv_qtile(0, 1)
        phase3(0, 1, zp1, False)
        proj_qk(2, 1)
        scores_head(4, 1)
        scores_head(5, 1)
        scores_last_pair(2, 1)
        masks_head(4, 1)
        masks_head(5, 1)
        zp2 = pv_qtile(0, 2)
        phase3(0, 2, zp2, False)
        for st in range(4, NST):
            proj_v(st)
        zp3 = pv_qtile(0, 3)
        phase3(0, 3, zp3, False)
        zp4 = pv_qtile(1, 0)
        zp5 = pv_qtile(1, 1)
        phase3(1, 0, zp4, True)
        phase3(1, 1, zp5, True)
        zp6 = pv_qtile(1, 2)
        phase3(1, 2, zp6, True)
        zp7 = pv_qtile(1, 3)
        phase3(1, 3, zp7, True)

    if not nc.is_finalized():
        nc.finalize()
    return nc


def _get_program():
    if "nc" not in _CACHE:
        _CACHE["nc"] = _build()
    return _CACHE["nc"]


F8NP = ml_dtypes.float8_e4m3
BFNP = ml_dtypes.bfloat16


def _q8(a):
    return a.astype(F8NP)


def make_in_maps(normalized_resid_pre, W_Q, W_K, W_V, b_Q):
    x = np.asarray(normalized_resid_pre, np.float32)
    W_Q = np.asarray(W_Q, np.float32) * FW
    W_K = np.asarray(W_K, np.float32) * FW
    W_V = np.asarray(W_V, np.float32) * FW
    b_Q = np.asarray(b_Q, np.float32) * FW

    tri = np.triu(np.ones((P, P), np.float32)).astype(BFNP)
    ident = np.eye(P, dtype=np.float32).astype(BFNP)
    trid = np.concatenate([tri, ident], axis=1)

    in_maps = []
    for c in range(8):
        b, hg = divmod(c, 2)
        hs = slice(hg * NHC, (hg + 1) * NHC)
        xt = np.ascontiguousarray(x[b].T)          # [D, S]
        xh = _q8(xt)
        xhf = xh.astype(np.float32)
        xls = _q8((xt - xhf) * 16.0)
        xhd = (xhf / 16.0).astype(F8NP)            # exact exponent shift

        wall = np.zeros((D, WALL_W), F8NP)
        for wname, W in (("q", W_Q), ("k", W_K), ("v", W_V)):
            Wg = W[hs].transpose(1, 0, 2).reshape(D, HD)  # [D, 6*64]
            Wh = _q8(Wg)
            Whf = Wh.astype(np.float32)
            Whd = (Whf / 16.0).astype(F8NP)
            Wls = _q8((Wg - Whf) * 16.0)
            if wname == "v":
                c0 = WALL_COLS["vh"][0]
                wall[:, c0 : c0 + HD] = Wh
                c0 = WALL_COLS["vhd"][0]
                wall[:, c0 : c0 + HD] = Whd
                c0 = WALL_COLS["vls"][0]
                wall[:, c0 : c0 + HD] = Wls
            else:
                for g in range(NPAIR):
                    sl = slice(g * P, (g + 1) * P)
                    c0 = WALL_COLS[wname + "h" + str(g)][0]
                    wall[:, c0 : c0 + P] = Wh[:, sl]
                    c0 = WALL_COLS[wname + "hd" + str(g)][0]
                    wall[:, c0 : c0 + P] = Whd[:, sl]
                    if QK_3T:
                        c0 = WALL_COLS[wname + "ls" + str(g)][0]
                        wall[:, c0 : c0 + P] = Wls[:, sl]

        in_maps.append({
            "wall": wall,
            "xh": xh,
            "xls": xls,
            "xhd": xhd,
            "wo": None,  # filled in kernel()
            "bq": np.ascontiguousarray(b_Q[hs].reshape(NPAIR, P).T),
            "trid": trid,
        })
    return in_maps


def kernel(
    normalized_resid_pre, W_Q, W_K, W_V, W_O, b_Q, b_K, b_V, b_O, **_unused
):
    W_O = np.asarray(W_O, np.float32)
    b_V, b_O = np.asarray(b_V, np.float32), np.asarray(b_O, np.float32)
    in_maps = make_in_maps(normalized_resid_pre, W_Q, W_K, W_V, b_Q)
    for c in range(8):
        hg = c % 2
        hs = slice(hg * NHC, (hg + 1) * NHC)
        in_maps[c]["wo"] = np.ascontiguousarray(
            (W_O[hs].reshape(HD, D) / FW).astype(BFNP))

    nc = _get_program()
    res = run_bass_kernel_spmd(nc, in_maps, list(range(8))).results

    out = np.zeros((B, S, D), np.float32)
    for c in range(8):
        out[c // 2] += res[c]["out"].astype(np.float32)
    out += b_O + np.einsum("nh,nhd->d", b_V, W_O)
    return out
